# revision 5
# baseline (speedup 1.0000x reference)
"""Trainium2 Bass kernel for Deimv2 LQE (softmax -> top4 -> stat -> MLP -> +scores).

Contract: kernel(**inputs) takes FULL unsharded numpy inputs
(scores [64,4096,1], pred_corners [64,4096,132], w1 [20,64], b1 [64],
w2 [64,1], b2 [1]) and returns the full [64,4096,1] float32 output.
Internally shards batch across 8 NeuronCores (pure data parallel).

Math notes:
  - softmax over 33 bins/corner is computed without max-subtraction
    (inputs are ~N(0,1); exp is safe in fp32).
  - top4(softmax(x)) == top4(exp(x)) / sum(exp(x)) by monotonicity; the
    sorted top-8 comes from the DVE Max instruction, we use lanes 0..3.
  - stat = [p1..p4, mean(p1..p4)] per corner (20 values/row), then
    h = relu(stat @ w1 + b1); q = h @ w2 + b2; out = scores + q.
    The MLP runs on the tensor engine with stat transposed via PE
    transpose; scores and b2 enter the final PSUM accumulation as extra
    rank-1 matmuls so the output DMAs straight from PSUM.
"""

import sys

for _p in ("/opt/trn_rl_repo", "/opt/trn_rl_repo/concourse"):
    if _p not in sys.path:
        sys.path.insert(0, _p)

import numpy as np

import concourse.bass as bass
import concourse.mybir as mybir
import concourse.tile as tile
from concourse import masks
from concourse.bass_utils import run_bass_kernel_spmd

# Problem shape (hardcoded per contract)
B, L = 64, 4096
NBINS1 = 33          # bins per corner
C = 4                # corners
D = C * NBINS1       # 132
TOPK = 4
HID = 64
IN_DIM = C * (TOPK + 1)  # 20
N_CORES = 8
R = B * L // N_CORES     # rows per core = 32768
G = R // 128             # row-groups per core = 256
T = 16                   # row-groups per tile
NTILES = G // T          # 16
GP = 32                  # padded stat dims per group (20 used + 12 zero)

F32 = mybir.dt.float32


def _split_waits(nc, max_waits=1):
    """This walrus build rejects instructions with >1 sync wait; move excess
    waits onto preceding same-engine NoOps (engine order preserves them)."""
    n_new = 0
    for f in nc.m.functions:
        for bb in f.blocks:
            insts = bb.instructions
            i = 0
            while i < len(insts):
                inst = insts[i]
                si = inst.sync_info
                if si is not None and len(si.on_wait) > max_waits:
                    waits = list(si.on_wait)
                    keep, rest = waits[:max_waits], waits[max_waits:]
                    nops = []
                    while rest:
                        chunk, rest = rest[:max_waits], rest[max_waits:]
                        nop = mybir.InstNoOp(
                            name=f"I-waitsplit-{n_new}", ins=[], outs=[]
                        )
                        n_new += 1
                        nop.engine = inst.engine
                        nop.sync_info = mybir.SyncInfo(on_wait=chunk, on_update=[])
                        nops.append(nop)
                    inst.sync_info = mybir.SyncInfo(
                        on_wait=keep, on_update=list(si.on_update)
                    )
                    for j, nop in enumerate(nops):
                        insts.insert(i + j, nop)
                    i += len(nops)
                i += 1
    return n_new


def build_kernel():
    nc = bass.Bass(trn_type="TRN2")
    pc = nc.dram_tensor("pc", [R, D], F32, kind="ExternalInput")
    sc = nc.dram_tensor("sc", [R, 1], F32, kind="ExternalInput")
    w1 = nc.dram_tensor("w1", [IN_DIM, HID], F32, kind="ExternalInput")
    b1 = nc.dram_tensor("b1", [HID, 1], F32, kind="ExternalInput")
    w2 = nc.dram_tensor("w2", [HID, 1], F32, kind="ExternalInput")
    b2 = nc.dram_tensor("b2", [1, 1], F32, kind="ExternalInput")
    out = nc.dram_tensor("out", [R, 1], F32, kind="ExternalOutput")

    with tile.TileContext(nc) as tc:
        with (
            tc.tile_pool(name="singles", bufs=1) as singles,
            tc.tile_pool(name="xin", bufs=3) as xin_pool,
            tc.tile_pool(name="ex", bufs=2) as ex_pool,
            tc.tile_pool(name="small", bufs=2) as small_pool,
            tc.tile_pool(name="stt", bufs=2) as stt_pool,
            tc.tile_pool(name="hsb", bufs=3) as hs_pool,
            tc.tile_pool(name="scin", bufs=3) as sc_pool,
            tc.tile_pool(name="pst", bufs=2, space="PSUM") as pst_psum,
            tc.tile_pool(name="ph", bufs=3, space="PSUM") as h_psum,
            tc.tile_pool(name="pq", bufs=3, space="PSUM") as q_psum,
        ):
            # ---- one-time constants ----
            ident = singles.tile([128, 128], F32)
            masks.make_identity(nc, ident[:])

            w1dA = singles.tile([128, 128], F32)
            w1dB = singles.tile([128, 128], F32)
            nc.gpsimd.memset(w1dA[:], 0.0)
            nc.gpsimd.memset(w1dB[:], 0.0)
            # block-sparse W1: group gg in the 4-group transpose batch sits at
            # stat rows gg*32..gg*32+20; groups (0,1)->A cols (0:64, 64:128),
            # groups (2,3)->B cols (0:64, 64:128)
            nc.sync.dma_start(out=w1dA[0:IN_DIM, 0:HID], in_=w1[:, :])
            nc.sync.dma_start(out=w1dA[32 : 32 + IN_DIM, HID:128], in_=w1[:, :])
            nc.sync.dma_start(out=w1dB[64 : 64 + IN_DIM, 0:HID], in_=w1[:, :])
            nc.sync.dma_start(out=w1dB[96 : 96 + IN_DIM, HID:128], in_=w1[:, :])

            b1_2 = singles.tile([128, 1], F32)
            nc.sync.dma_start(out=b1_2[0:HID, :], in_=b1[:, :])
            nc.sync.dma_start(out=b1_2[HID:128, :], in_=b1[:, :])

            w2d = singles.tile([128, 2], F32)
            nc.gpsimd.memset(w2d[:], 0.0)
            nc.sync.dma_start(out=w2d[0:HID, 0:1], in_=w2[:, :])
            nc.sync.dma_start(out=w2d[HID:128, 1:2], in_=w2[:, :])

            b2c1 = singles.tile([1, 2], F32)
            nc.sync.dma_start(out=b2c1[0:1, 0:1], in_=b2[:, :])
            nc.sync.dma_start(out=b2c1[0:1, 1:2], in_=b2[:, :])

            ones1 = singles.tile([1, 512], F32)
            nc.gpsimd.memset(ones1[:], 1.0)

            eye2 = singles.tile([2, 2], F32)
            masks.make_identity(nc, eye2[:])

            # ---- main loop over tiles of T row-groups ----
            for it in range(NTILES):
                base = it * T * 128  # first row of tile
                x = xin_pool.tile([128, T * D], F32, tag="x")
                nc.sync.dma_start(
                    out=x[:].rearrange("p (g d) -> p g d", d=D),
                    in_=pc[base : base + T * 128, :].rearrange(
                        "(g p) d -> p g d", p=128
                    ),
                )

                e = ex_pool.tile([128, T * D], F32, tag="e")
                nc.scalar.activation(
                    out=e[:], in_=x[:], func=mybir.ActivationFunctionType.Exp
                )
                e3 = e[:].rearrange("p (tc nb) -> p tc nb", nb=NBINS1)  # [128,64,33]

                z = small_pool.tile([128, T * C], F32, tag="z")
                nc.vector.reduce_sum(out=z[:], in_=e3, axis=mybir.AxisListType.X)
                r = small_pool.tile([128, T * C], F32, tag="r")
                nc.vector.reciprocal(out=r[:], in_=z[:])

                t8 = small_pool.tile([128, T * C, 8], F32, tag="t8")
                for tcb in range(T * C):
                    nc.vector.max(out=t8[:, tcb, :], in_=e3[:, tcb, :])

                m4 = small_pool.tile([128, T * C], F32, tag="m4")
                nc.vector.reduce_sum(
                    out=m4[:], in_=t8[:, :, 0:4], axis=mybir.AxisListType.X
                )

                # stat: [128, T, 32] padded; cols c*5+s hold p_s, col c*5+4 mean
                stat = small_pool.tile([128, T, GP], F32, tag="stat")
                nc.gpsimd.memset(stat[:, :, IN_DIM:GP], 0.0)
                statc = stat[:, :, 0:IN_DIM].rearrange(
                    "p g (c s) -> p g c s", s=TOPK + 1
                )
                r3 = r[:].rearrange("p (g c) -> p g c", c=C)
                nc.vector.tensor_mul(
                    statc[:, :, :, 0:TOPK],
                    t8[:, :, 0:TOPK].rearrange("p (g c) k -> p g c k", c=C),
                    r3.to_broadcast([128, T, C, TOPK]),
                )
                nc.vector.scalar_tensor_tensor(
                    out=statc[:, :, :, TOPK],
                    in0=m4[:].rearrange("p (g c) -> p g c", c=C),
                    scalar=1.0 / TOPK,
                    in1=r3,
                    op0=mybir.AluOpType.mult,
                    op1=mybir.AluOpType.mult,
                )

                # 4 transpose batches of 4 groups -> statT [128,128] each
                for hb in range(2):  # half-tiles of 8 groups / 1024 rows
                    hT = h_psum.tile([128, 512], F32, tag="hT")
                    for bb in range(2):  # 4-group transpose batch in half-tile
                        bt = hb * 2 + bb
                        sT_ps = pst_psum.tile([128, 128], F32, tag="sT")
                        nc.tensor.transpose(
                            out=sT_ps[:],
                            in_=stat[:, bt * 4 : (bt + 1) * 4, :].rearrange(
                                "p g k -> p (g k)"
                            ),
                            identity=ident[:],
                        )
                        sT = stt_pool.tile([128, 128], F32, tag="sT_sb")
                        nc.scalar.copy(out=sT[:], in_=sT_ps[:])
                        nc.tensor.matmul(
                            out=hT[:, (bb * 2) * 128 : (bb * 2 + 1) * 128],
                            lhsT=w1dA[:],
                            rhs=sT[:],
                        )
                        nc.tensor.matmul(
                            out=hT[:, (bb * 2 + 1) * 128 : (bb * 2 + 2) * 128],
                            lhsT=w1dB[:],
                            rhs=sT[:],
                        )
                    hs = hs_pool.tile([128, 512], F32, tag="hs")
                    nc.scalar.activation(
                        out=hs[:],
                        in_=hT[:],
                        func=mybir.ActivationFunctionType.Relu,
                        bias=b1_2[:],
                    )

                    hbase = base + hb * 1024
                    scT = sc_pool.tile([2, 4, 128], F32, tag="scT")
                    nc.sync.dma_start(
                        out=scT[:],
                        in_=sc[hbase : hbase + 1024, :].rearrange(
                            "(q two p) one -> two q (p one)", two=2, p=128
                        ),
                    )
                    q = q_psum.tile([2, 512], F32, tag="q")
                    nc.tensor.matmul(
                        out=q[:], lhsT=w2d[:], rhs=hs[:], start=True, stop=False
                    )
                    nc.tensor.matmul(
                        out=q[:], lhsT=b2c1[:], rhs=ones1[:], start=False, stop=False
                    )
                    nc.tensor.matmul(
                        out=q[:],
                        lhsT=eye2[:],
                        rhs=scT[:].rearrange("t q p -> t (q p)"),
                        start=False,
                        stop=True,
                    )
                    q_sb = sc_pool.tile([2, 512], F32, tag="q_sb")
                    nc.scalar.copy(out=q_sb[:], in_=q[:])
                    nc.sync.dma_start(
                        out=out[hbase : hbase + 1024, :].rearrange(
                            "(q two p) one -> two q (p one)", two=2, p=128
                        ),
                        in_=q_sb[:].rearrange("t (q p) -> t q p", p=128),
                    )

    _split_waits(nc)
    return nc


_CACHE = {}


def kernel(scores, pred_corners, w1, b1, w2, b2):
    if "nc" not in _CACHE:
        _CACHE["nc"] = build_kernel()
    nc = _CACHE["nc"]

    pc_full = np.ascontiguousarray(
        pred_corners.reshape(B * L, D).astype(np.float32, copy=False)
    )
    sc_full = np.ascontiguousarray(
        scores.reshape(B * L, 1).astype(np.float32, copy=False)
    )
    w1_a = np.ascontiguousarray(w1.astype(np.float32, copy=False))
    b1_a = np.ascontiguousarray(b1.astype(np.float32, copy=False)).reshape(HID, 1)
    w2_a = np.ascontiguousarray(w2.astype(np.float32, copy=False)).reshape(HID, 1)
    b2_a = np.ascontiguousarray(b2.astype(np.float32, copy=False)).reshape(1, 1)

    in_maps = []
    for c in range(N_CORES):
        in_maps.append(
            {
                "pc": pc_full[c * R : (c + 1) * R],
                "sc": sc_full[c * R : (c + 1) * R],
                "w1": w1_a,
                "b1": b1_a,
                "w2": w2_a,
                "b2": b2_a,
            }
        )
    res = run_bass_kernel_spmd(nc, in_maps, core_ids=list(range(N_CORES)))
    outs = [res.results[c]["out"] for c in range(N_CORES)]
    full = np.concatenate(outs, axis=0).reshape(B, L, 1)
    return full


# revision 17
# speedup vs baseline: 8715.5821x; 8715.5821x over previous
"""Trainium2 Bass kernel for Deimv2 LQE (softmax -> top4 -> stat -> MLP -> +scores).

Contract: kernel(**inputs) takes FULL unsharded numpy inputs
(scores [64,4096,1], pred_corners [64,4096,132], w1 [20,64], b1 [64],
w2 [64,1], b2 [1]) and returns the full [64,4096,1] float32 output.
Internally shards batch across 8 NeuronCores (pure data parallel).

Math notes:
  - softmax over 33 bins/corner is computed without max-subtraction
    (inputs are ~N(0,1); exp is safe in fp32).
  - top4(softmax(x)) == top4(exp(x)) / sum(exp(x)) by monotonicity; the
    sorted top-8 comes from the DVE Max instruction, we use lanes 0..3.
  - stat = [p1..p4, mean(p1..p4)] per corner (20 values/row), then
    h = relu(stat @ w1 + b1); q = h @ w2 + b2; out = scores + q.
    The mean slot is linear in p1..p4 so it is folded into W1 on-chip
    (w1'[c*5+i] = w1[c*5+i] + w1[c*5+4]/4) and the MLP sees zeros there.
    The MLP runs on the tensor engine with stat transposed via PE
    transpose; scores and b2 enter the final PSUM accumulation as extra
    rank-1 matmuls, then one ACT copy moves q to SBUF for the output DMA.
  - The emission order is software-pipelined (DMA i+2, exp i+1, compute i)
    because the in-order ACT queue otherwise serializes exp behind the
    previous tile's PSUM copies.
"""

import sys

for _p in ("/opt/trn_rl_repo", "/opt/trn_rl_repo/concourse"):
    if _p not in sys.path:
        sys.path.insert(0, _p)

import numpy as np

import concourse.bass as bass
import concourse.mybir as mybir
import concourse.tile as tile
from concourse import masks
from concourse.bass_utils import run_bass_kernel_spmd

# Problem shape (hardcoded per contract)
B, L = 64, 4096
NBINS1 = 33          # bins per corner
C = 4                # corners
D = C * NBINS1       # 132
TOPK = 4
HID = 64
IN_DIM = C * (TOPK + 1)  # 20
N_CORES = 8
R = B * L // N_CORES     # rows per core = 32768
G = R // 128             # row-groups per core = 256
T = 16                   # row-groups per tile
NTILES = G // T          # 16
GP = 32                  # padded stat dims per group (20 used + 12 zero)

F32 = mybir.dt.float32


def _split_waits(nc, max_waits=1):
    """This walrus build rejects instructions with >1 sync wait; move excess
    waits onto preceding same-engine NoOps (engine order preserves them)."""
    n_new = 0
    for f in nc.m.functions:
        for bb in f.blocks:
            insts = bb.instructions
            i = 0
            while i < len(insts):
                inst = insts[i]
                si = inst.sync_info
                if si is not None and len(si.on_wait) > max_waits:
                    waits = list(si.on_wait)
                    keep, rest = waits[:max_waits], waits[max_waits:]
                    nops = []
                    while rest:
                        chunk, rest = rest[:max_waits], rest[max_waits:]
                        nop = mybir.InstNoOp(
                            name=f"I-waitsplit-{n_new}", ins=[], outs=[]
                        )
                        n_new += 1
                        nop.engine = inst.engine
                        nop.sync_info = mybir.SyncInfo(on_wait=chunk, on_update=[])
                        nops.append(nop)
                    inst.sync_info = mybir.SyncInfo(
                        on_wait=keep, on_update=list(si.on_update)
                    )
                    for j, nop in enumerate(nops):
                        insts.insert(i + j, nop)
                    i += len(nops)
                i += 1
    return n_new


def build_kernel(niter=1, ablate=frozenset()):
    nc = bass.Bass(trn_type="TRN2")
    pc = nc.dram_tensor("pc", [R, D], F32, kind="ExternalInput")
    sc = nc.dram_tensor("sc", [R, 1], F32, kind="ExternalInput")
    w1 = nc.dram_tensor("w1", [IN_DIM, HID], F32, kind="ExternalInput")
    b1 = nc.dram_tensor("b1", [HID, 1], F32, kind="ExternalInput")
    w2 = nc.dram_tensor("w2", [HID, 1], F32, kind="ExternalInput")
    b2 = nc.dram_tensor("b2", [1, 1], F32, kind="ExternalInput")
    out = nc.dram_tensor("out", [R, 1], F32, kind="ExternalOutput")

    with tile.TileContext(nc) as tc:
        with (
            tc.tile_pool(name="singles", bufs=1) as singles,
            tc.tile_pool(name="xin", bufs=3) as xin_pool,
            tc.tile_pool(name="ex", bufs=2) as ex_pool,
            tc.tile_pool(name="small", bufs=2) as small_pool,
            tc.tile_pool(name="stt", bufs=2) as stt_pool,
            tc.tile_pool(name="hsb", bufs=3) as hs_pool,
            tc.tile_pool(name="scin", bufs=3) as sc_pool,
            tc.tile_pool(name="pst", bufs=2, space="PSUM") as pst_psum,
            tc.tile_pool(name="ph", bufs=3, space="PSUM") as h_psum,
            tc.tile_pool(name="pq", bufs=3, space="PSUM") as q_psum,
        ):
            # ---- one-time constants ----
            ident = singles.tile([128, 128], F32)
            masks.make_identity(nc, ident[:])

            # The mean stat slot is linear in p1..p4, so fold it into W1:
            # w1'[c*5+i] = w1[c*5+i] + w1[c*5+4]/4 and feed mean-slot zeros.
            # This deletes the top4-mean reduce + multiply from DVE.
            w1eff = singles.tile([IN_DIM, HID], F32)
            w1mean = singles.tile([IN_DIM, HID], F32)
            nc.sync.dma_start(out=w1eff[:, :], in_=w1[:, :])
            for c in range(C):
                mrow = w1[c * (TOPK + 1) + TOPK, :]  # [64]
                bc = bass.AP(
                    tensor=mrow.tensor,
                    offset=mrow.offset,
                    ap=[[0, TOPK + 1], [1, HID]],
                )
                nc.sync.dma_start(
                    out=w1mean[c * (TOPK + 1) : (c + 1) * (TOPK + 1), :], in_=bc
                )
            nc.vector.scalar_tensor_tensor(
                out=w1eff[:, :],
                in0=w1mean[:, :],
                scalar=1.0 / TOPK,
                in1=w1eff[:, :],
                op0=mybir.AluOpType.mult,
                op1=mybir.AluOpType.add,
            )

            w1dA = singles.tile([128, 128], F32)
            w1dB = singles.tile([128, 128], F32)
            nc.gpsimd.memset(w1dA[:], 0.0)
            nc.gpsimd.memset(w1dB[:], 0.0)
            # block-sparse W1: group gg in the 4-group transpose batch sits at
            # stat rows gg*32..gg*32+20; groups (0,1)->A cols (0:64, 64:128),
            # groups (2,3)->B cols (0:64, 64:128)
            nc.sync.dma_start(out=w1dA[0:IN_DIM, 0:HID], in_=w1eff[:, :])
            nc.sync.dma_start(out=w1dA[32 : 32 + IN_DIM, HID:128], in_=w1eff[:, :])
            nc.sync.dma_start(out=w1dB[64 : 64 + IN_DIM, 0:HID], in_=w1eff[:, :])
            nc.sync.dma_start(out=w1dB[96 : 96 + IN_DIM, HID:128], in_=w1eff[:, :])

            b1_2 = singles.tile([128, 1], F32)
            nc.sync.dma_start(out=b1_2[0:HID, :], in_=b1[:, :])
            nc.sync.dma_start(out=b1_2[HID:128, :], in_=b1[:, :])

            w2d = singles.tile([128, 2], F32)
            nc.gpsimd.memset(w2d[:], 0.0)
            nc.sync.dma_start(out=w2d[0:HID, 0:1], in_=w2[:, :])
            nc.sync.dma_start(out=w2d[HID:128, 1:2], in_=w2[:, :])

            b2c1 = singles.tile([1, 2], F32)
            nc.sync.dma_start(out=b2c1[0:1, 0:1], in_=b2[:, :])
            nc.sync.dma_start(out=b2c1[0:1, 1:2], in_=b2[:, :])

            ones1 = singles.tile([1, 512], F32)
            nc.gpsimd.memset(ones1[:], 1.0)

            eye2 = singles.tile([2, 2], F32)
            masks.make_identity(nc, eye2[:])

            # ---- software-pipelined main loop ----
            # ACT executes in order: exp(i+1) must be emitted BEFORE tile i's
            # PSUM copies/relus (which wait on tile i's DVE+PE) or every exp
            # serializes the whole pipeline behind the previous tile.
            x_tiles = {}
            e_tiles = {}

            def dma_tile(it):
                x = xin_pool.tile([128, T * D], F32, tag="x")
                x_tiles[it] = x
                if "dma" not in ablate:
                    nc.sync.dma_start(
                        out=x[:].rearrange("p (g d) -> p g d", d=D),
                        in_=pc[it * T * 128 : (it + 1) * T * 128, :].rearrange(
                            "(g p) d -> p g d", p=128
                        ),
                    )
                else:
                    nc.gpsimd.memset(x[:, 0:1], 0.0)

            def exp_tile(it):
                x = x_tiles.pop(it)
                if "exp" not in ablate:
                    e = ex_pool.tile([128, T * D], F32, tag="e")
                    nc.scalar.activation(
                        out=e[:], in_=x[:], func=mybir.ActivationFunctionType.Exp
                    )
                    e_tiles[it] = e
                else:
                    e_tiles[it] = x

            def compute_tile(it):
                base = it * T * 128
                e = e_tiles.pop(it)
                e3 = e[:].rearrange("p (tc nb) -> p tc nb", nb=NBINS1)  # [128,64,33]

                z = small_pool.tile([128, T * C], F32, tag="z")
                nc.vector.reduce_sum(out=z[:], in_=e3, axis=mybir.AxisListType.X)
                r = small_pool.tile([128, T * C], F32, tag="r")
                nc.vector.reciprocal(out=r[:], in_=z[:])

                if "max" not in ablate:
                    t8 = small_pool.tile([128, T * C, 8], F32, tag="t8")
                    for tcb in range(T * C):
                        nc.vector.max(out=t8[:, tcb, :], in_=e3[:, tcb, :])
                    t8v = t8[:, :, 0:4]
                else:
                    t8v = e3[:, :, 0:4]

                # stat: [128, T, 32]; cols c*5+s hold p_s, mean slots + pad
                # stay zero (mean folded into w1eff)
                stat = small_pool.tile([128, T, GP], F32, tag="stat")
                nc.gpsimd.memset(stat[:], 0.0)
                statc = stat[:, :, 0:IN_DIM].rearrange(
                    "p g (c s) -> p g c s", s=TOPK + 1
                )
                r3 = r[:].rearrange("p (g c) -> p g c", c=C)
                nc.vector.tensor_mul(
                    statc[:, :, :, 0:TOPK],
                    t8v.rearrange("p (g c) k -> p g c k", c=C),
                    r3.to_broadcast([128, T, C, TOPK]),
                )

                # 4 transposes into one PSUM bank, ONE psum->sbuf copy
                sT_ps = pst_psum.tile([128, 512], F32, tag="sT")
                for bt in range(4):
                    nc.tensor.transpose(
                        out=sT_ps[:, bt * 128 : (bt + 1) * 128],
                        in_=stat[:, bt * 4 : (bt + 1) * 4, :].rearrange(
                            "p g k -> p (g k)"
                        ),
                        identity=ident[:],
                    )
                sT = stt_pool.tile([128, 512], F32, tag="sT_sb")
                nc.scalar.copy(out=sT[:], in_=sT_ps[:])

                for hb in range(2):  # half-tiles of 8 groups / 1024 rows
                    hbase = base + hb * 1024
                    hT = h_psum.tile([128, 512], F32, tag="hT")
                    for bb in range(2):
                        bt = hb * 2 + bb
                        nc.tensor.matmul(
                            out=hT[:, (bb * 2) * 128 : (bb * 2 + 1) * 128],
                            lhsT=w1dA[:],
                            rhs=sT[:, bt * 128 : (bt + 1) * 128],
                        )
                        nc.tensor.matmul(
                            out=hT[:, (bb * 2 + 1) * 128 : (bb * 2 + 2) * 128],
                            lhsT=w1dB[:],
                            rhs=sT[:, bt * 128 : (bt + 1) * 128],
                        )
                    hs = hs_pool.tile([128, 512], F32, tag="hs")
                    nc.scalar.activation(
                        out=hs[:],
                        in_=hT[:],
                        func=mybir.ActivationFunctionType.Relu,
                        bias=b1_2[:],
                    )

                    scT = sc_pool.tile([2, 4, 128], F32, tag="scT")
                    nc.sync.dma_start(
                        out=scT[:],
                        in_=sc[hbase : hbase + 1024, :].rearrange(
                            "(q two p) one -> two q (p one)", two=2, p=128
                        ),
                    )
                    q = q_psum.tile([2, 512], F32, tag="q")
                    nc.tensor.matmul(
                        out=q[:], lhsT=w2d[:], rhs=hs[:], start=True, stop=False
                    )
                    nc.tensor.matmul(
                        out=q[:], lhsT=b2c1[:], rhs=ones1[:], start=False, stop=False
                    )
                    nc.tensor.matmul(
                        out=q[:],
                        lhsT=eye2[:],
                        rhs=scT[:].rearrange("t q p -> t (q p)"),
                        start=False,
                        stop=True,
                    )
                    q_sb = sc_pool.tile([2, 512], F32, tag="q_sb")
                    nc.scalar.copy(out=q_sb[:], in_=q[:])
                    nc.sync.dma_start(
                        out=out[hbase : hbase + 1024, :].rearrange(
                            "(q two p) one -> two q (p one)", two=2, p=128
                        ),
                        in_=q_sb[:].rearrange("t (q p) -> t q p", p=128),
                    )

            def staged_tiles():
                dma_tile(0)
                if NTILES > 1:
                    dma_tile(1)
                exp_tile(0)
                for i in range(NTILES):
                    if i + 2 < NTILES:
                        dma_tile(i + 2)
                    if i + 1 < NTILES:
                        exp_tile(i + 1)
                    compute_tile(i)

            if niter == 1:
                staged_tiles()
            else:
                # hardware loop: body emitted once, executed niter times
                # (bench-only path for on-device timing via slope)
                with tc.For_i(0, niter, 1):
                    staged_tiles()

    _split_waits(nc)
    return nc


_CACHE = {}


def kernel(scores, pred_corners, w1, b1, w2, b2):
    if "nc" not in _CACHE:
        _CACHE["nc"] = build_kernel()
    nc = _CACHE["nc"]

    pc_full = np.ascontiguousarray(
        pred_corners.reshape(B * L, D).astype(np.float32, copy=False)
    )
    sc_full = np.ascontiguousarray(
        scores.reshape(B * L, 1).astype(np.float32, copy=False)
    )
    w1_a = np.ascontiguousarray(w1.astype(np.float32, copy=False))
    b1_a = np.ascontiguousarray(b1.astype(np.float32, copy=False)).reshape(HID, 1)
    w2_a = np.ascontiguousarray(w2.astype(np.float32, copy=False)).reshape(HID, 1)
    b2_a = np.ascontiguousarray(b2.astype(np.float32, copy=False)).reshape(1, 1)

    in_maps = []
    for c in range(N_CORES):
        in_maps.append(
            {
                "pc": pc_full[c * R : (c + 1) * R],
                "sc": sc_full[c * R : (c + 1) * R],
                "w1": w1_a,
                "b1": b1_a,
                "w2": w2_a,
                "b2": b2_a,
            }
        )
    res = run_bass_kernel_spmd(nc, in_maps, core_ids=list(range(N_CORES)))
    outs = [res.results[c]["out"] for c in range(N_CORES)]
    full = np.concatenate(outs, axis=0).reshape(B, L, 1)
    return full
